# revision 9
# baseline (speedup 1.0000x reference)
"""Trainium2 distributed kernel for the AppearanceReconstruction loss.

Math note (exact identity, not an approximation): the MAE shuffle/gather in
the reference collapses — restored[b,p] is appearance_tokens[b,p] on kept
slots (which the mask multiplies by zero) and mask_token on masked slots.
Every row has exactly num_masked = 2 masked slots, and the decoder output at
a masked slot is the same single vector r = MLP(LN(mask_token)) for all
(b,p). Hence

    loss = 2 * sum_b mean_c((r_c - pooled[b,c])^2) / (256 + 1e-8)
    pooled[b] = mean_n target_features[b,n,:]

The memory-bound part (reading all of target_features, 402 MB) runs on the 8
NeuronCores, data-parallel over B (16 rows per core). Each core streams its
50 MB shard through SBUF in 3 MB tiles and reduces over N with TensorEngine
matmuls against one-hot columns (scaled by 1/N), accumulating the 16 row
means into a [16, 768] PSUM tile. A short vector-engine epilogue computes
sum_c (pooled - r)^2 per row; the host sums the 8x16 partials.
"""

import math

import numpy as np

B, N, C = 128, 1024, 768
NCORES = 8
BPC = B // NCORES  # rows per core
PPB = 128  # SBUF partitions per row-tile
NSUB = N // PPB  # n-rows folded into each partition's free dim
FREE = NSUB * C  # 6144 floats per partition per row-tile
LN_EPS = 1e-5

_CACHE = {}


def _build():
    import concourse.bass as bass  # noqa: F401
    import concourse.tile as tile
    from concourse import bacc, mybir

    f32 = mybir.dt.float32
    f32r = mybir.dt.float32r
    AL = mybir.AluOpType
    AX = mybir.AxisListType

    nc = bacc.Bacc(
        "TRN2", target_bir_lowering=False, debug=False, num_devices=NCORES
    )
    tf = nc.dram_tensor("tf", [BPC, PPB, FREE], f32r, kind="ExternalInput")
    rrep = nc.dram_tensor("rrep", [BPC, C], f32, kind="ExternalInput")
    emat = nc.dram_tensor("emat", [PPB, BPC * BPC], f32r, kind="ExternalInput")
    out = nc.dram_tensor("out", [BPC, 2], f32, kind="ExternalOutput")

    with tile.TileContext(nc) as tc:
        with (
            tc.tile_pool(name="consts", bufs=1) as cpool,
            tc.tile_pool(name="data", bufs=6) as dpool,
            tc.tile_pool(name="epi", bufs=1) as epool,
            tc.tile_pool(name="psum", bufs=1, space="PSUM") as ppool,
        ):
            emat_sb = cpool.tile([PPB, BPC * BPC], f32r)
            nc.sync.dma_start(out=emat_sb[:], in_=emat.ap())
            rrep_sb = cpool.tile([BPC, C], f32)
            nc.sync.dma_start(out=rrep_sb[:], in_=rrep.ap())

            psA = ppool.tile([BPC, 512], f32)
            psB = ppool.tile([BPC, 256], f32)

            for b in range(BPC):
                t = dpool.tile([PPB, FREE], f32r, tag="data")
                nc.sync.dma_start(out=t[:], in_=tf.ap()[b])
                # float32r: same 4-byte layout, 4x faster PE streaming; the
                # reduced-precision multiply is far inside the loss tolerance.
                lhsT = emat_sb[:, b * BPC : (b + 1) * BPC]
                first = b == 0
                last = b == BPC - 1
                for sub in range(NSUB):
                    nc.tensor.matmul(
                        psA[:],
                        lhsT,
                        t[:, sub * C : sub * C + 512],
                        start=first and sub == 0,
                        stop=last and sub == NSUB - 1,
                    )
                for sub in range(NSUB):
                    nc.tensor.matmul(
                        psB[:],
                        lhsT,
                        t[:, sub * C + 512 : (sub + 1) * C],
                        start=first and sub == 0,
                        stop=last and sub == NSUB - 1,
                    )

            d = epool.tile([BPC, C], f32)
            nc.vector.tensor_tensor(
                out=d[:, 0:512], in0=psA[:], in1=rrep_sb[:, 0:512], op=AL.subtract
            )
            nc.vector.tensor_tensor(
                out=d[:, 512:768], in0=psB[:], in1=rrep_sb[:, 512:768], op=AL.subtract
            )
            sq = epool.tile([BPC, C], f32)
            nc.vector.tensor_tensor(out=sq[:], in0=d[:], in1=d[:], op=AL.mult)
            s = epool.tile([BPC, 2], f32)
            nc.vector.tensor_reduce(
                out=s[:, 0:1], in_=sq[:, 0:512], axis=AX.X, op=AL.add
            )
            nc.vector.tensor_reduce(
                out=s[:, 1:2], in_=sq[:, 512:768], axis=AX.X, op=AL.add
            )
            nc.sync.dma_start(out=out.ap(), in_=s[:])

    nc.compile()
    return nc


def _get_nc():
    nc = _CACHE.get("nc")
    if nc is None:
        nc = _build()
        _CACHE["nc"] = nc
    return nc


def _host_r(mask_token, ln_w, ln_b, W1, b1, W2, b2):
    """r = Linear2(gelu_exact(Linear1(LayerNorm(mask_token)))) — one 768-vec."""
    mt = np.asarray(mask_token, np.float64).reshape(C)
    mu = mt.mean()
    var = ((mt - mu) ** 2).mean()
    x = (mt - mu) / np.sqrt(var + LN_EPS) * np.asarray(ln_w, np.float64) + np.asarray(
        ln_b, np.float64
    )
    h = x @ np.asarray(W1, np.float64) + np.asarray(b1, np.float64)
    erf = np.frompyfunc(math.erf, 1, 1)
    g = h * 0.5 * (1.0 + erf(h / math.sqrt(2.0)).astype(np.float64))
    r = g @ np.asarray(W2, np.float64) + np.asarray(b2, np.float64)
    return r.astype(np.float32)


def kernel(
    appearance_tokens,
    target_features,
    noise,
    mask_token,
    ln_w,
    ln_b,
    W1,
    b1,
    W2,
    b2,
):
    from concourse.bass_utils import run_bass_kernel_spmd

    nc = _get_nc()

    r = _host_r(mask_token, ln_w, ln_b, W1, b1, W2, b2)
    rrep = np.ascontiguousarray(np.broadcast_to(r, (BPC, C)), np.float32)

    # emat[:, b*16+m] = 1/N if m == b else 0 — one-hot columns scaled so the
    # partition-reduction matmul lands mean_n directly in PSUM row b.
    emat = np.zeros((PPB, BPC * BPC), np.float32)
    for b in range(BPC):
        emat[:, b * BPC + b] = 1.0 / N

    tfull = np.ascontiguousarray(target_features, np.float32).reshape(
        NCORES, BPC, PPB, FREE
    )
    in_maps = [
        {"tf": tfull[i], "rrep": rrep, "emat": emat} for i in range(NCORES)
    ]

    res = run_bass_kernel_spmd(nc, in_maps, list(range(NCORES)))
    total = 0.0
    for i in range(NCORES):
        total += float(np.asarray(res.results[i]["out"], np.float64).sum())

    loss = 2.0 * total / C / (256.0 + 1e-8)
    return np.float32(loss)


# revision 12
# speedup vs baseline: 1.6909x; 1.6909x over previous
"""Trainium2 distributed kernel for the AppearanceReconstruction loss.

Math note (exact identity, not an approximation): the MAE shuffle/gather in
the reference collapses — restored[b,p] is appearance_tokens[b,p] on kept
slots (which the mask multiplies by zero) and mask_token on masked slots.
Every row has exactly num_masked = 2 masked slots, and the decoder output at
a masked slot is the same single vector r = MLP(LN(mask_token)) for all
(b,p). Hence

    loss = 2 * sum_b mean_c((r_c - pooled[b,c])^2) / (256 + 1e-8)
    pooled[b] = mean_n target_features[b,n,:]

The memory-bound part (reading all of target_features, 402 MB) runs on the 8
NeuronCores, data-parallel over B (16 rows per core). Each core streams its
50 MB shard through SBUF in 3 MB tiles and reduces over N with TensorEngine
matmuls against one-hot columns (scaled by 1/N), accumulating the 16 row
means into a [16, 768] PSUM tile. A short vector-engine epilogue computes
sum_c (pooled - r)^2 per row; the host sums the 8x16 partials.
"""

import math

import numpy as np

B, N, C = 128, 1024, 768
NCORES = 8
BPC = B // NCORES  # rows per core
PPB = 128  # SBUF partitions per row-tile
NSUB = N // PPB  # n-rows folded into each partition's free dim
FREE = NSUB * C  # 6144 floats per partition per row-tile
LN_EPS = 1e-5

_CACHE = {}


def _build():
    import concourse.bass as bass  # noqa: F401
    import concourse.tile as tile
    from concourse import bacc, mybir

    f32 = mybir.dt.float32
    f32r = mybir.dt.float32r
    AL = mybir.AluOpType
    AX = mybir.AxisListType

    nc = bacc.Bacc(
        "TRN2", target_bir_lowering=False, debug=False, num_devices=NCORES
    )
    tf = nc.dram_tensor("tf", [BPC, PPB, FREE], f32r, kind="ExternalInput")
    rrep = nc.dram_tensor("rrep", [BPC, C], f32, kind="ExternalInput")
    emat = nc.dram_tensor("emat", [PPB, BPC * BPC], f32r, kind="ExternalInput")
    out = nc.dram_tensor("out", [BPC, 2], f32, kind="ExternalOutput")

    with tile.TileContext(nc) as tc:
        with (
            tc.tile_pool(name="consts", bufs=1) as cpool,
            tc.tile_pool(name="data", bufs=6) as dpool,
            tc.tile_pool(name="epi", bufs=1) as epool,
            tc.tile_pool(name="psum", bufs=1, space="PSUM") as ppool,
        ):
            # issue the first data chunk's DMA before the tiny const loads so
            # the big stream starts as early as possible; the first row is
            # split in half to ramp the DMA queue with multiple transfers
            half = NSUB // 2
            t0_tile = dpool.tile([PPB, half * C], f32r, tag="data")
            nc.sync.dma_start(out=t0_tile[:], in_=tf.ap()[0, :, 0 : half * C])

            emat_sb = cpool.tile([PPB, BPC * BPC], f32r)
            nc.sync.dma_start(out=emat_sb[:], in_=emat.ap())
            rrep_sb = cpool.tile([BPC, C], f32)
            nc.sync.dma_start(out=rrep_sb[:], in_=rrep.ap())

            # two row-groups with separate PSUM accumulators: group 0 (rows
            # 0-7) finishes mid-stream and its epilogue overlaps the DMA of
            # later rows; only group 1's epilogue sits after the last DMA
            G = BPC // 2
            ps = [
                (
                    ppool.tile([G, 512], f32, name=f"psA{g}", tag=f"psA{g}"),
                    ppool.tile([G, 256], f32, name=f"psB{g}", tag=f"psB{g}"),
                )
                for g in range(2)
            ]

            # (row, sub_lo, sub_hi) chunks in row order; the last row ends
            # with quarter-chunks so the post-final-DMA PE tail is tiny
            chunks = [(0, 0, half), (0, half, NSUB)]
            chunks += [(b, 0, NSUB) for b in range(1, BPC - 1)]
            chunks += [
                (BPC - 1, 0, half),
                (BPC - 1, half, half + NSUB // 4),
                (BPC - 1, half + NSUB // 4, NSUB),
            ]
            grp_first = {0: (0, 0), 1: (8, 0)}  # group -> (row, sub) of start
            grp_last = {0: (7, NSUB - 1), 1: (BPC - 1, NSUB - 1)}

            def epilogue(g):
                psA, psB = ps[g]
                d = epool.tile([G, C], f32, tag=f"d{g}")
                nc.vector.tensor_tensor(
                    out=d[:, 0:512],
                    in0=psA[:],
                    in1=rrep_sb[0:G, 0:512],
                    op=AL.subtract,
                )
                nc.vector.tensor_tensor(
                    out=d[:, 512:768],
                    in0=psB[:],
                    in1=rrep_sb[0:G, 512:768],
                    op=AL.subtract,
                )
                sq = epool.tile([G, C], f32, tag=f"sq{g}")
                nc.vector.tensor_tensor(out=sq[:], in0=d[:], in1=d[:], op=AL.mult)
                s = epool.tile([G, 2], f32, tag=f"s{g}")
                nc.vector.tensor_reduce(
                    out=s[:, 0:1], in_=sq[:, 0:512], axis=AX.X, op=AL.add
                )
                nc.vector.tensor_reduce(
                    out=s[:, 1:2], in_=sq[:, 512:768], axis=AX.X, op=AL.add
                )
                nc.sync.dma_start(out=out.ap()[g * G : (g + 1) * G, :], in_=s[:])

            for ci, (b, lo, hi) in enumerate(chunks):
                if ci == 0:
                    t = t0_tile
                else:
                    t = dpool.tile([PPB, (hi - lo) * C], f32r, tag="data")
                    nc.sync.dma_start(
                        out=t[:], in_=tf.ap()[b, :, lo * C : hi * C]
                    )
                g = b // G
                psA, psB = ps[g]
                # float32r: same 4-byte layout, 4x faster PE streaming; the
                # reduced-precision multiply is far inside the loss tolerance.
                # One-hot column (absolute index 17*b) lands row b in PSUM
                # partition b - 8*g of its group's accumulator.
                lhsT = emat_sb[:, b * BPC + g * G : b * BPC + g * G + G]
                for sub in range(lo, hi):
                    nc.tensor.matmul(
                        psA[:],
                        lhsT,
                        t[:, (sub - lo) * C : (sub - lo) * C + 512],
                        start=(b, sub) == grp_first[g],
                        stop=(b, sub) == grp_last[g],
                    )
                for sub in range(lo, hi):
                    nc.tensor.matmul(
                        psB[:],
                        lhsT,
                        t[:, (sub - lo) * C + 512 : (sub - lo + 1) * C],
                        start=(b, sub) == grp_first[g],
                        stop=(b, sub) == grp_last[g],
                    )
                if (b, hi - 1) == grp_last[g]:
                    epilogue(g)

    nc.compile()
    return nc


def _get_nc():
    nc = _CACHE.get("nc")
    if nc is None:
        nc = _build()
        _CACHE["nc"] = nc
    return nc


def _host_r(mask_token, ln_w, ln_b, W1, b1, W2, b2):
    """r = Linear2(gelu_exact(Linear1(LayerNorm(mask_token)))) — one 768-vec."""
    mt = np.asarray(mask_token, np.float64).reshape(C)
    mu = mt.mean()
    var = ((mt - mu) ** 2).mean()
    x = (mt - mu) / np.sqrt(var + LN_EPS) * np.asarray(ln_w, np.float64) + np.asarray(
        ln_b, np.float64
    )
    h = x @ np.asarray(W1, np.float64) + np.asarray(b1, np.float64)
    erf = np.frompyfunc(math.erf, 1, 1)
    g = h * 0.5 * (1.0 + erf(h / math.sqrt(2.0)).astype(np.float64))
    r = g @ np.asarray(W2, np.float64) + np.asarray(b2, np.float64)
    return r.astype(np.float32)


def kernel(
    appearance_tokens,
    target_features,
    noise,
    mask_token,
    ln_w,
    ln_b,
    W1,
    b1,
    W2,
    b2,
):
    from concourse.bass_utils import run_bass_kernel_spmd

    nc = _get_nc()

    r = _host_r(mask_token, ln_w, ln_b, W1, b1, W2, b2)
    rrep = np.ascontiguousarray(np.broadcast_to(r, (BPC, C)), np.float32)

    # emat[:, b*16+m] = 1/N if m == b else 0 — one-hot columns scaled so the
    # partition-reduction matmul lands mean_n directly in PSUM row b.
    emat = np.zeros((PPB, BPC * BPC), np.float32)
    for b in range(BPC):
        emat[:, b * BPC + b] = 1.0 / N

    tfull = np.ascontiguousarray(target_features, np.float32).reshape(
        NCORES, BPC, PPB, FREE
    )
    in_maps = [
        {"tf": tfull[i], "rrep": rrep, "emat": emat} for i in range(NCORES)
    ]

    res = run_bass_kernel_spmd(nc, in_maps, list(range(NCORES)))
    total = 0.0
    for i in range(NCORES):
        total += float(np.asarray(res.results[i]["out"], np.float64).sum())

    loss = 2.0 * total / C / (256.0 + 1e-8)
    return np.float32(loss)


# revision 13
# speedup vs baseline: 1.8771x; 1.1101x over previous
"""Trainium2 distributed kernel for the AppearanceReconstruction loss.

Math note (exact identity, not an approximation): the MAE shuffle/gather in
the reference collapses — restored[b,p] is appearance_tokens[b,p] on kept
slots (which the mask multiplies by zero) and mask_token on masked slots.
Every row has exactly num_masked = 2 masked slots, and the decoder output at
a masked slot is the same single vector r = MLP(LN(mask_token)) for all
(b,p). Hence

    loss = 2 * sum_b mean_c((r_c - pooled[b,c])^2) / (256 + 1e-8)
    pooled[b] = mean_n target_features[b,n,:]

The memory-bound part (reading all of target_features, 402 MB) runs on the 8
NeuronCores, data-parallel over B (16 rows per core). Each core streams its
50 MB shard through SBUF in 3 MB tiles and reduces over N with TensorEngine
matmuls against one-hot columns (scaled by 1/N), accumulating the 16 row
means into a [16, 768] PSUM tile. A short vector-engine epilogue computes
sum_c (pooled - r)^2 per row; the host sums the 8x16 partials.
"""

import math

import numpy as np

B, N, C = 128, 1024, 768
NCORES = 8
BPC = B // NCORES  # rows per core
PPB = 128  # SBUF partitions per row-tile
NSUB = N // PPB  # n-rows folded into each partition's free dim
FREE = NSUB * C  # 6144 floats per partition per row-tile
LN_EPS = 1e-5

_CACHE = {}


def _build():
    import concourse.bass as bass  # noqa: F401
    import concourse.tile as tile
    from concourse import bacc, mybir

    f32 = mybir.dt.float32
    f32r = mybir.dt.float32r
    AL = mybir.AluOpType
    AX = mybir.AxisListType

    nc = bacc.Bacc(
        "TRN2", target_bir_lowering=False, debug=False, num_devices=NCORES
    )
    tf = nc.dram_tensor("tf", [BPC, PPB, FREE], f32r, kind="ExternalInput")
    rrep = nc.dram_tensor("rrep", [BPC, C], f32, kind="ExternalInput")
    emat = nc.dram_tensor("emat", [PPB, BPC * BPC], f32r, kind="ExternalInput")
    out = nc.dram_tensor("out", [BPC, 2], f32, kind="ExternalOutput")

    with tile.TileContext(nc) as tc:
        with (
            tc.tile_pool(name="consts", bufs=1) as cpool,
            tc.tile_pool(name="data", bufs=6) as dpool,
            tc.tile_pool(name="epi", bufs=1) as epool,
            tc.tile_pool(name="psum", bufs=1, space="PSUM") as ppool,
        ):
            # issue the first data tile's DMA before the tiny const loads so
            # the 3 MB stream starts as early as possible
            t0_tile = dpool.tile([PPB, FREE], f32r, tag="data")
            nc.sync.dma_start(out=t0_tile[:], in_=tf.ap()[0])

            emat_sb = cpool.tile([PPB, BPC * BPC], f32r)
            nc.sync.dma_start(out=emat_sb[:], in_=emat.ap())
            rrep_sb = cpool.tile([BPC, C], f32)
            nc.sync.dma_start(out=rrep_sb[:], in_=rrep.ap())

            psA = ppool.tile([BPC, 512], f32)
            psB = ppool.tile([BPC, 256], f32)

            # (row, sub_lo, sub_hi) chunks; full 3 MB rows keep the DMA
            # stream at peak rate, only the last row is halved so the
            # post-final-DMA PE tail is half a row
            half = NSUB // 2
            chunks = [(b, 0, NSUB) for b in range(BPC - 1)]
            chunks += [(BPC - 1, 0, half), (BPC - 1, half, NSUB)]

            for ci, (b, lo, hi) in enumerate(chunks):
                if ci == 0:
                    t = t0_tile
                else:
                    t = dpool.tile([PPB, (hi - lo) * C], f32r, tag="data")
                    nc.sync.dma_start(
                        out=t[:], in_=tf.ap()[b, :, lo * C : hi * C]
                    )
                # float32r: same 4-byte layout, 4x faster PE streaming; the
                # reduced-precision multiply is far inside the loss tolerance.
                lhsT = emat_sb[:, b * BPC : (b + 1) * BPC]
                first = ci == 0
                last = ci == len(chunks) - 1
                for sub in range(lo, hi):
                    nc.tensor.matmul(
                        psA[:],
                        lhsT,
                        t[:, (sub - lo) * C : (sub - lo) * C + 512],
                        start=first and sub == lo,
                        stop=last and sub == hi - 1,
                    )
                for sub in range(lo, hi):
                    nc.tensor.matmul(
                        psB[:],
                        lhsT,
                        t[:, (sub - lo) * C + 512 : (sub - lo + 1) * C],
                        start=first and sub == lo,
                        stop=last and sub == hi - 1,
                    )

            d = epool.tile([BPC, C], f32)
            nc.vector.tensor_tensor(
                out=d[:, 0:512], in0=psA[:], in1=rrep_sb[:, 0:512], op=AL.subtract
            )
            nc.vector.tensor_tensor(
                out=d[:, 512:768], in0=psB[:], in1=rrep_sb[:, 512:768], op=AL.subtract
            )
            sq = epool.tile([BPC, C], f32)
            nc.vector.tensor_tensor(out=sq[:], in0=d[:], in1=d[:], op=AL.mult)
            s = epool.tile([BPC, 2], f32)
            nc.vector.tensor_reduce(
                out=s[:, 0:1], in_=sq[:, 0:512], axis=AX.X, op=AL.add
            )
            nc.vector.tensor_reduce(
                out=s[:, 1:2], in_=sq[:, 512:768], axis=AX.X, op=AL.add
            )
            # output DMA on the ACT HWDGE ring so it never queues behind the
            # SP ring's bulk data stream
            nc.scalar.dma_start(out=out.ap(), in_=s[:])

    nc.compile()
    return nc


def _get_nc():
    nc = _CACHE.get("nc")
    if nc is None:
        nc = _build()
        _CACHE["nc"] = nc
    return nc


def _host_r(mask_token, ln_w, ln_b, W1, b1, W2, b2):
    """r = Linear2(gelu_exact(Linear1(LayerNorm(mask_token)))) — one 768-vec."""
    mt = np.asarray(mask_token, np.float64).reshape(C)
    mu = mt.mean()
    var = ((mt - mu) ** 2).mean()
    x = (mt - mu) / np.sqrt(var + LN_EPS) * np.asarray(ln_w, np.float64) + np.asarray(
        ln_b, np.float64
    )
    h = x @ np.asarray(W1, np.float64) + np.asarray(b1, np.float64)
    erf = np.frompyfunc(math.erf, 1, 1)
    g = h * 0.5 * (1.0 + erf(h / math.sqrt(2.0)).astype(np.float64))
    r = g @ np.asarray(W2, np.float64) + np.asarray(b2, np.float64)
    return r.astype(np.float32)


def kernel(
    appearance_tokens,
    target_features,
    noise,
    mask_token,
    ln_w,
    ln_b,
    W1,
    b1,
    W2,
    b2,
):
    from concourse.bass_utils import run_bass_kernel_spmd

    nc = _get_nc()

    r = _host_r(mask_token, ln_w, ln_b, W1, b1, W2, b2)
    rrep = np.ascontiguousarray(np.broadcast_to(r, (BPC, C)), np.float32)

    # emat[:, b*16+m] = 1/N if m == b else 0 — one-hot columns scaled so the
    # partition-reduction matmul lands mean_n directly in PSUM row b.
    emat = np.zeros((PPB, BPC * BPC), np.float32)
    for b in range(BPC):
        emat[:, b * BPC + b] = 1.0 / N

    tfull = np.ascontiguousarray(target_features, np.float32).reshape(
        NCORES, BPC, PPB, FREE
    )
    in_maps = [
        {"tf": tfull[i], "rrep": rrep, "emat": emat} for i in range(NCORES)
    ]

    res = run_bass_kernel_spmd(nc, in_maps, list(range(NCORES)))
    total = 0.0
    for i in range(NCORES):
        total += float(np.asarray(res.results[i]["out"], np.float64).sum())

    loss = 2.0 * total / C / (256.0 + 1e-8)
    return np.float32(loss)


# revision 16
# speedup vs baseline: 1.8834x; 1.0034x over previous
"""Trainium2 distributed kernel for the AppearanceReconstruction loss.

Math note (exact identity, not an approximation): the MAE shuffle/gather in
the reference collapses — restored[b,p] is appearance_tokens[b,p] on kept
slots (which the mask multiplies by zero) and mask_token on masked slots.
Every row has exactly num_masked = 2 masked slots, and the decoder output at
a masked slot is the same single vector r = MLP(LN(mask_token)) for all
(b,p). Hence

    loss = 2 * sum_b mean_c((r_c - pooled[b,c])^2) / (256 + 1e-8)
    pooled[b] = mean_n target_features[b,n,:]

The memory-bound part (reading all of target_features, 402 MB) runs on the 8
NeuronCores, data-parallel over B (16 rows per core). Each core streams its
50 MB shard through SBUF in 3 MB tiles and reduces over N with TensorEngine
matmuls against one-hot columns (scaled by 1/N), accumulating the 16 row
means into a [16, 768] PSUM tile. A short vector-engine epilogue computes
sum_c (pooled - r)^2 per row; the host sums the 8x16 partials.
"""

import math

import numpy as np

B, N, C = 128, 1024, 768
NCORES = 8
BPC = B // NCORES  # rows per core
PPB = 128  # SBUF partitions per row-tile
NSUB = N // PPB  # n-rows folded into each partition's free dim
FREE = NSUB * C  # 6144 floats per partition per row-tile
LN_EPS = 1e-5

_CACHE = {}

# kernel structure knobs (A/B-tested on hardware; defaults = best measured)
_VARIANT = {"last_split": True, "out_ring": "scalar"}


def _build():
    import concourse.bass as bass  # noqa: F401
    import concourse.tile as tile
    from concourse import bacc, mybir

    f32 = mybir.dt.float32
    f32r = mybir.dt.float32r
    AL = mybir.AluOpType
    AX = mybir.AxisListType

    nc = bacc.Bacc(
        "TRN2", target_bir_lowering=False, debug=False, num_devices=NCORES
    )
    tf = nc.dram_tensor("tf", [BPC, PPB, FREE], f32r, kind="ExternalInput")
    rrep = nc.dram_tensor("rrep", [BPC, C], f32, kind="ExternalInput")
    emat = nc.dram_tensor("emat", [PPB, BPC * BPC], f32r, kind="ExternalInput")
    out = nc.dram_tensor("out", [BPC, 2], f32, kind="ExternalOutput")

    with tile.TileContext(nc) as tc:
        with (
            tc.tile_pool(name="consts", bufs=1) as cpool,
            tc.tile_pool(name="data", bufs=6) as dpool,
            tc.tile_pool(name="epi", bufs=1) as epool,
            tc.tile_pool(name="psum", bufs=1, space="PSUM") as ppool,
        ):
            # issue the first data tile's DMA before the tiny const loads so
            # the 3 MB stream starts as early as possible
            t0_tile = dpool.tile([PPB, FREE], f32r, tag="data")
            nc.sync.dma_start(out=t0_tile[:], in_=tf.ap()[0])

            emat_sb = cpool.tile([PPB, BPC * BPC], f32r)
            nc.sync.dma_start(out=emat_sb[:], in_=emat.ap())
            rrep_sb = cpool.tile([BPC, C], f32)
            nc.sync.dma_start(out=rrep_sb[:], in_=rrep.ap())

            psA = ppool.tile([BPC, 512], f32)
            psB = ppool.tile([BPC, 256], f32)

            # (row, sub_lo, sub_hi) chunks; full 3 MB rows keep the DMA
            # stream at peak rate, only the last row is halved so the
            # post-final-DMA PE tail is half a row
            half = NSUB // 2
            if _VARIANT["last_split"]:
                chunks = [(b, 0, NSUB) for b in range(BPC - 1)]
                chunks += [(BPC - 1, 0, half), (BPC - 1, half, NSUB)]
            else:
                chunks = [(b, 0, NSUB) for b in range(BPC)]

            for ci, (b, lo, hi) in enumerate(chunks):
                if ci == 0:
                    t = t0_tile
                else:
                    t = dpool.tile([PPB, (hi - lo) * C], f32r, tag="data")
                    nc.sync.dma_start(
                        out=t[:], in_=tf.ap()[b, :, lo * C : hi * C]
                    )
                # float32r: same 4-byte layout, 4x faster PE streaming; the
                # reduced-precision multiply is far inside the loss tolerance.
                lhsT = emat_sb[:, b * BPC : (b + 1) * BPC]
                first = ci == 0
                last = ci == len(chunks) - 1
                for sub in range(lo, hi):
                    nc.tensor.matmul(
                        psA[:],
                        lhsT,
                        t[:, (sub - lo) * C : (sub - lo) * C + 512],
                        start=first and sub == lo,
                        stop=last and sub == hi - 1,
                    )
                for sub in range(lo, hi):
                    nc.tensor.matmul(
                        psB[:],
                        lhsT,
                        t[:, (sub - lo) * C + 512 : (sub - lo + 1) * C],
                        start=first and sub == lo,
                        stop=last and sub == hi - 1,
                    )

            d = epool.tile([BPC, C], f32)
            nc.vector.tensor_tensor(
                out=d[:, 0:512], in0=psA[:], in1=rrep_sb[:, 0:512], op=AL.subtract
            )
            nc.vector.tensor_tensor(
                out=d[:, 512:768], in0=psB[:], in1=rrep_sb[:, 512:768], op=AL.subtract
            )
            sq = epool.tile([BPC, C], f32)
            nc.vector.tensor_tensor(out=sq[:], in0=d[:], in1=d[:], op=AL.mult)
            s = epool.tile([BPC, 2], f32)
            nc.vector.tensor_reduce(
                out=s[:, 0:1], in_=sq[:, 0:512], axis=AX.X, op=AL.add
            )
            nc.vector.tensor_reduce(
                out=s[:, 1:2], in_=sq[:, 512:768], axis=AX.X, op=AL.add
            )
            # output DMA on the ACT HWDGE ring so it never queues behind the
            # SP ring's bulk data stream
            out_eng = nc.scalar if _VARIANT["out_ring"] == "scalar" else nc.sync
            out_eng.dma_start(out=out.ap(), in_=s[:])

    nc.compile()
    return nc


def _get_nc():
    nc = _CACHE.get("nc")
    if nc is None:
        nc = _build()
        _CACHE["nc"] = nc
    return nc


def _host_r(mask_token, ln_w, ln_b, W1, b1, W2, b2):
    """r = Linear2(gelu_exact(Linear1(LayerNorm(mask_token)))) — one 768-vec."""
    mt = np.asarray(mask_token, np.float64).reshape(C)
    mu = mt.mean()
    var = ((mt - mu) ** 2).mean()
    x = (mt - mu) / np.sqrt(var + LN_EPS) * np.asarray(ln_w, np.float64) + np.asarray(
        ln_b, np.float64
    )
    h = x @ np.asarray(W1, np.float64) + np.asarray(b1, np.float64)
    erf = np.frompyfunc(math.erf, 1, 1)
    g = h * 0.5 * (1.0 + erf(h / math.sqrt(2.0)).astype(np.float64))
    r = g @ np.asarray(W2, np.float64) + np.asarray(b2, np.float64)
    return r.astype(np.float32)


def kernel(
    appearance_tokens,
    target_features,
    noise,
    mask_token,
    ln_w,
    ln_b,
    W1,
    b1,
    W2,
    b2,
):
    from concourse.bass_utils import run_bass_kernel_spmd

    nc = _get_nc()

    r = _host_r(mask_token, ln_w, ln_b, W1, b1, W2, b2)
    rrep = np.ascontiguousarray(np.broadcast_to(r, (BPC, C)), np.float32)

    # emat[:, b*16+m] = 1/N if m == b else 0 — one-hot columns scaled so the
    # partition-reduction matmul lands mean_n directly in PSUM row b.
    emat = np.zeros((PPB, BPC * BPC), np.float32)
    for b in range(BPC):
        emat[:, b * BPC + b] = 1.0 / N

    tfull = np.ascontiguousarray(target_features, np.float32).reshape(
        NCORES, BPC, PPB, FREE
    )
    in_maps = [
        {"tf": tfull[i], "rrep": rrep, "emat": emat} for i in range(NCORES)
    ]

    res = run_bass_kernel_spmd(nc, in_maps, list(range(NCORES)))
    total = 0.0
    for i in range(NCORES):
        total += float(np.asarray(res.results[i]["out"], np.float64).sum())

    loss = 2.0 * total / C / (256.0 + 1e-8)
    return np.float32(loss)
